# revision 41
# baseline (speedup 1.0000x reference)
"""Block-causal sparse attention (MLA latent KV + GQA + RoPE) on 8 TRN2 cores.

Sharding: 2 batches x 4 query-chunks of 512 tokens (T-sharding). Each core
computes its 512 output rows end-to-end (q/kv projections, sparse attention,
o-projection) over a gathered key set of 768 window rows + 32 global slots.
Everything runs in a transposed layout (feature dim on partitions): the host
supplies x^T slices and the kernel returns out^T.

Attention stage: scores are computed directly in S^T layout (keys on
partitions, queries on free dim), so P^T = exp(S^T) feeds the P@V matmul
with NO PE transposes and no per-(head,l) DVE normalize/copy chains.
Masking is a 0/1 multiply after exp (strict-lower-triangle diag mask +
per-chunk globals mask). Softmax sums come from a [128,128] ones-matrix
stationary matmul over P^T — the output arrives pre-broadcast on all 128
partitions, so normalization is one reciprocal_approx_fast + one DVE
multiply on y^T per head with no cross-engine broadcast (avoids a
PE->DVE->GpSimd->ACT semaphore convoy).

Other perf-relevant choices (each validated against a perfetto trace):
- all matmul operands bf16 (fp32 PSUM accumulation)
- x^T resident in SBUF; stage A (kv-down) streams it chunk-by-chunk and
  C1 (q-proj) reads it in place — no DVE staging copies
- Wq/Wo are pre-transposed on the host into per-chunk-contiguous blocks:
  a strided on-the-fly rearrange DMA costs ~4.2us/chunk (256B elements)
  vs ~0.6us contiguous, and the stalls re-throttle the PE clock (HAM)
- DMA priority order on one HWDGE queue: x/wkv, first 3 wq chunks, rope
  tables, wku/wvu + masks, remaining wq (pool-paced), all of Wo
  (prefetched during B/C2), outputs
- C1 and C2 are software-pipelined one head deep; stage B interleaves
  V-tile matmuls into the K rope chain to keep the PE dense
- PSUM is budgeted to exactly 8 banks in C2 (AB x2, C, G, y, sums)
"""

import functools
import numpy as np
import ml_dtypes

BF16 = ml_dtypes.bfloat16

# Model constants (hardcoded per problem spec)
D = 2048        # d_model
L = 512         # MLA latent
NH = 16         # query heads
NKV = 4         # kv heads
HD = 128        # head dim
B, T = 2, 2048
BLOCK = 128
WINDOW = 256
GEV = 64        # global every
THETA = 10000.0

# Sharding geometry
NCORES = 8
TQ = 512        # queries per core
KWIN = 768      # window key rows (t0-256 .. t0+512)
NG = 32         # global slots (padded)
KT = 896        # key layout: 768 window | 32 global | 96 zero pad
KQ0 = 256       # query cols inside key layout
SCALE = 1.0 / float(np.sqrt(HD))

# S^T PSUM packing: (kw, psum tag, col offset, q range)
#   tag AB = [128,1024]: kw0@0 (q0:128) | kw2@128 (q0:384) | kw1@512 (q0:256)
#                        | kw4@768 (q256:512)
#   tag C  = [128,512]:  kw3@0 (q128:512) | kw5@384 (q384:512)
#   tag G  = [128,512]:  globals (q0:512)
# pT SBUF mirror: [128, 4, 512] bf16; slot0/1 = AB, slot2 = C, slot3 = G.
KW_LAYOUT = [
    # kw, slot, col0, q0, q1
    (0, 0, 0,   0,   128),
    (2, 0, 128, 0,   384),
    (1, 1, 0,   0,   256),
    (4, 1, 256, 256, 512),
    (3, 2, 0,   128, 512),
    (5, 2, 384, 384, 512),
]
# diag tiles (kw == l): (slot, col offset in slot, l)
DIAG_SLICES = [(0, 0, 0), (1, 128, 1), (0, 384, 2), (2, 256, 3)]


def _build_program(loop_n=None):
    import contextlib
    import concourse.bacc as bacc
    import concourse.tile as tile
    import concourse.mybir as mybir

    f32 = mybir.dt.float32
    bf = mybir.dt.bfloat16
    EXP = mybir.ActivationFunctionType.Exp
    CPY = mybir.ActivationFunctionType.Copy

    nc = bacc.Bacc("TRN2", target_bir_lowering=False, debug=False)

    xT = nc.dram_tensor("xT", [D, KT], bf, kind="ExternalInput")
    wq = nc.dram_tensor("wq", [NH, 128, 16, HD], bf, kind="ExternalInput")
    wkv = nc.dram_tensor("wkv", [D, L], bf, kind="ExternalInput")
    wku = nc.dram_tensor("wku", [L, NKV * HD], bf, kind="ExternalInput")
    wvu = nc.dram_tensor("wvu", [L, NKV * HD], bf, kind="ExternalInput")
    wo = nc.dram_tensor("wo", [16, 128, 16, HD], bf, kind="ExternalInput")
    cosT = nc.dram_tensor("cosT", [HD, KT], f32, kind="ExternalInput")
    sinT = nc.dram_tensor("sinT", [HD, KT], f32, kind="ExternalInput")
    rotM = nc.dram_tensor("rotM", [HD, HD], bf, kind="ExternalInput")
    gmask = nc.dram_tensor("gmask", [128, TQ], bf, kind="ExternalInput")
    wones = nc.dram_tensor("wones", [128, 7, 128], bf, kind="ExternalInput")
    diag01 = nc.dram_tensor("diag01", [128, 128], bf, kind="ExternalInput")
    outT = nc.dram_tensor("outT", [D, TQ], f32, kind="ExternalOutput")

    def act_copy(out, in_):
        nc.scalar.activation(out, in_, CPY)

    with tile.TileContext(nc) as tc, contextlib.ExitStack() as _es:
        if loop_n:
            _es.enter_context(tc.For_i(0, loop_n, 1))
        with tc.tile_pool(name="const", bufs=1) as constp:
            qT_all = constp.tile([HD, NH, TQ], bf)       # roped q^T per head
            warm = constp.tile([1, 2], f32)
            nc.vector.memset(warm[:], 0.0)
            act_warm = constp.tile([1, 2], f32)
            nc.scalar.activation(act_warm[:], warm[:],
                                 mybir.ActivationFunctionType.Exp)
            kT_sb = constp.tile([HD, NKV, KT], bf)       # roped K^T per kv head
            V_sb = constp.tile([128, 7, NKV * HD], bf)   # V rows x (kv*hd)
            yT_sb = constp.tile([HD, NH, TQ], bf)        # attention out^T
            wo_all = constp.tile([128, 16, 16, HD], bf)  # all of Wo^T, chunked

            with tc.tile_pool(name="ckvp", bufs=1) as ckvp:
                ckv_sb = ckvp.tile([128, 4, KT], bf)
                wku_sb = ckvp.tile([128, 4, NKV * HD], bf)
                wvu_sb = ckvp.tile([128, 4, NKV * HD], bf)

                with tc.tile_pool(name="xp", bufs=1) as xp, \
                     tc.tile_pool(name="wkvp", bufs=8) as wkvp:
                    xfull = xp.tile([128, 16, KT], bf)   # x^T resident
                    # x^T/wkv DMAs first — stage A blocks on chunk 0 and
                    # consumes chunk k as it lands; everything else queues
                    # behind in priority order (see module docstring).
                    wkv_k = [None] * 16
                    for k in range(16):
                        wkv_k[k] = wkvp.tile([128, L], bf, tag="wkv",
                                             name=f"wkv{k % 8}")
                        if k == 0:
                            # first matmul blocks on wkv0 + x0's first half:
                            # load those first, in the smallest pieces
                            nc.sync.dma_start(wkv_k[0][:], wkv[0:128, :])
                            nc.sync.dma_start(xfull[:, 0, 0:512],
                                              xT[0:128, 0:512])
                            nc.sync.dma_start(xfull[:, 0, 512:KT],
                                              xT[0:128, 512:KT])
                        else:
                            nc.sync.dma_start(xfull[:, k, :],
                                              xT[k * 128:(k + 1) * 128, :])
                            nc.sync.dma_start(wkv_k[k][:],
                                              wkv[k * 128:(k + 1) * 128, :])

                    # ---- Stage A: c_kv^T = Wkv_down^T @ x^T -> [L, KT] ----
                    with tc.tile_pool(name="psA", bufs=1, space="PSUM") as psA:
                        ps_ckv = [psA.tile([128, KT], f32, tag=f"ckv{lt}",
                                           name=f"ckv{lt}")
                                  for lt in range(4)]
                        for k in range(16):
                            for lt in range(4):
                                for c0, c1 in ((0, 512), (512, 800)):
                                    nc.tensor.matmul(
                                        ps_ckv[lt][:, c0:c1],
                                        wkv_k[k][:, lt * 128:(lt + 1) * 128],
                                        xfull[:, k, c0:c1],
                                        start=(k == 0), stop=(k == 15),
                                    )
                        # cols 800:896 are key-layout padding: zero instead of
                        # computing (downstream K/V then see exact zeros)
                        for lt in range(4):
                            if lt % 2 == 0:
                                nc.vector.tensor_copy(ckv_sb[:, lt, 0:800],
                                                      ps_ckv[lt][:, 0:800])
                            else:
                                act_copy(ckv_sb[:, lt, 0:800],
                                         ps_ckv[lt][:, 0:800])
                            nc.vector.memset(ckv_sb[:, lt, 800:KT], 0.0)

                    # ---- Stage C1: q projection + RoPE for all heads ----
                    # DMA priority order: first 3 wq chunks right behind x
                    # (C1's first matmuls), then the small rope constants,
                    # then the remaining wq chunks paced by the pool.
                    with tc.tile_pool(name="wqp", bufs=4) as wqp, \
                         tc.tile_pool(name="tmpQ", bufs=2) as tmpQ, \
                         tc.tile_pool(name="psq", bufs=2, space="PSUM") as psq, \
                         tc.tile_pool(name="psr", bufs=2, space="PSUM") as psr:
                        wq_pre = []
                        for h in range(3):
                            wq_t = wqp.tile([128, 16, HD], bf, tag="wqh",
                                            name=f"wq{h % 4}")
                            nc.sync.dma_start(wq_t[:], wq[h])
                            wq_pre.append(wq_t)
                        cos_sb = constp.tile([HD, KT], f32)
                        nc.sync.dma_start(cos_sb[:], cosT[:])
                        sin_sb = constp.tile([HD, KT], f32)
                        nc.sync.dma_start(sin_sb[:], sinT[:])
                        rot_sb = constp.tile([HD, HD], bf)
                        nc.sync.dma_start(rot_sb[:], rotM[:])
                        for lk in range(4):
                            nc.sync.dma_start(wku_sb[:, lk, :],
                                              wku[lk * 128:(lk + 1) * 128, :])
                            nc.sync.dma_start(wvu_sb[:, lk, :],
                                              wvu[lk * 128:(lk + 1) * 128, :])
                        gmask_sb = constp.tile([128, TQ], bf)
                        nc.sync.dma_start(gmask_sb[:], gmask[:])
                        wones_sb = constp.tile([128, 7, 128], bf)
                        nc.sync.dma_start(wones_sb[:], wones[:])
                        diag_sb = constp.tile([128, 128], bf)
                        nc.sync.dma_start(diag_sb[:], diag01[:])
                        # software pipeline: head h's rope rotation matmul
                        # is emitted after head h+1's projection matmuls so the
                        # PE never waits on the ACT psum->bf16 copy
                        pend = [None] * NH

                        def q_proj(h, wq_h):
                            ps_q = psq.tile([128, TQ], f32, tag="q",
                                            name=f"q{h % 2}")
                            for k in range(16):
                                nc.tensor.matmul(
                                    ps_q[:], wq_h[:, k, :],
                                    xfull[:, k, KQ0:KQ0 + TQ],
                                    start=(k == 0), stop=(k == 15),
                                )
                            qh_r = tmpQ.tile([128, TQ], bf, tag="qhr",
                                             name=f"qhr{h % 2}")
                            act_copy(qh_r[:], ps_q[:])
                            t_qc = tmpQ.tile([128, TQ], f32, tag="tqc",
                                             name=f"tqc{h % 2}")
                            nc.vector.tensor_mul(t_qc[:], ps_q[:],
                                                 cos_sb[:, KQ0:KQ0 + TQ])
                            pend[h] = (qh_r, t_qc)

                        def q_rope(h):
                            qh_r, t_qc = pend[h]
                            ps_rt = psr.tile([128, TQ], f32, tag="qrot")
                            nc.tensor.matmul(ps_rt[:], rot_sb[:], qh_r[:],
                                             start=True, stop=True)
                            qt1 = tmpQ.tile([128, TQ], f32, tag="qt1")
                            nc.vector.tensor_mul(qt1[:], ps_rt[:],
                                                 sin_sb[:, KQ0:KQ0 + TQ])
                            nc.vector.tensor_add(qT_all[:, h, :], t_qc[:],
                                                 qt1[:])

                        for h in range(NH):
                            if h < 3:
                                wq_h = wq_pre[h]
                            else:
                                wq_h = wqp.tile([128, 16, HD], bf, tag="wqh",
                                                name=f"wq{h % 4}")
                                nc.sync.dma_start(wq_h[:], wq[h])
                            q_proj(h, wq_h)
                            if h > 0:
                                q_rope(h - 1)
                        q_rope(NH - 1)

                # Wo prefetch: queued on sync behind wq, runs during B/C2.
                for od in range(16):
                    nc.sync.dma_start(wo_all[:, od, :, :], wo[od])

                # ---- Stage B: K^T (roped) and V, interleaved for overlap ----
                with tc.tile_pool(name="tmpB", bufs=2) as tmpB, \
                     tc.tile_pool(name="pskh", bufs=2, space="PSUM") as pskh, \
                     tc.tile_pool(name="psrot", bufs=1, space="PSUM") as psrot, \
                     tc.tile_pool(name="psv", bufs=2, space="PSUM") as psv:
                    def k_head(g):
                        ps_kh = pskh.tile([128, KT], f32, tag="kh")
                        for lk in range(4):
                            for c0, c1 in ((0, 512), (512, 800)):
                                nc.tensor.matmul(
                                    ps_kh[:, c0:c1],
                                    wku_sb[:, lk, g * 128:(g + 1) * 128],
                                    ckv_sb[:, lk, c0:c1],
                                    start=(lk == 0), stop=(lk == 3),
                                )
                        return ps_kh

                    def k_rope(g, ps_kh):
                        # only the 800 real key columns; kT pad is memset so
                        # the globals-tile stationary reads exact zeros
                        kh_r = tmpB.tile([128, KT], bf, tag="khr")
                        act_copy(kh_r[:, 0:800], ps_kh[:, 0:800])
                        t_kc = tmpB.tile([128, KT], f32, tag="tkc")
                        nc.vector.tensor_mul(t_kc[:, 0:800], ps_kh[:, 0:800],
                                             cos_sb[:, 0:800])
                        ps_rot = psrot.tile([128, KT], f32, tag="rot")
                        for c0, c1 in ((0, 512), (512, 800)):
                            nc.tensor.matmul(ps_rot[:, c0:c1], rot_sb[:],
                                             kh_r[:, c0:c1], start=True, stop=True)
                        t1 = tmpB.tile([128, KT], f32, tag="t1")
                        nc.vector.tensor_mul(t1[:, 0:800], ps_rot[:, 0:800],
                                             sin_sb[:, 0:800])
                        nc.vector.tensor_add(kT_sb[:, g, 0:800], t_kc[:, 0:800],
                                             t1[:, 0:800])
                        nc.vector.memset(kT_sb[:, g, 800:KT], 0.0)

                    def v_tile(tt):
                        ps_v = psv.tile([128, 512], f32, tag="v")
                        for lk in range(4):
                            nc.tensor.matmul(
                                ps_v[:],
                                ckv_sb[:, lk, tt * 128:(tt + 1) * 128],
                                wvu_sb[:, lk, :],
                                start=(lk == 0), stop=(lk == 3),
                            )
                        if tt % 2 == 0:
                            nc.vector.tensor_copy(V_sb[:, tt, :], ps_v[:])
                        else:
                            act_copy(V_sb[:, tt, :], ps_v[:])

                    # interleave: V matmuls keep the PE busy while the DVE
                    # rope chain for each kv head drains
                    ps = k_head(0); v_tile(0)
                    k_rope(0, ps); v_tile(1)
                    ps = k_head(1); v_tile(2)
                    k_rope(1, ps); v_tile(3)
                    ps = k_head(2); v_tile(4)
                    k_rope(2, ps)
                    ps = k_head(3); v_tile(5)
                    k_rope(3, ps); v_tile(6)

            # ---- Stage C2: sparse attention in S^T layout, software-
            # pipelined over heads (PV/sums/norm of head h-1 run between
            # head h's scores and its exp/mask chain) ----
            with tc.tile_pool(name="pTp", bufs=3) as pTp, \
                 tc.tile_pool(name="nrm", bufs=2) as nrm, \
                 tc.tile_pool(name="psAB", bufs=2, space="PSUM") as psAB, \
                 tc.tile_pool(name="psC", bufs=1, space="PSUM") as psC, \
                 tc.tile_pool(name="psG", bufs=1, space="PSUM") as psG, \
                 tc.tile_pool(name="psY", bufs=1, space="PSUM") as psY, \
                 tc.tile_pool(name="psS", bufs=1, space="PSUM") as psS:

                pT_tiles = [None] * NH

                def scores(h):
                    g = h // 4
                    ab = psAB.tile([128, 1024], f32, tag="ab", name="ab")
                    cc = psC.tile([128, 512], f32, tag="c", name="c")
                    gg = psG.tile([128, 512], f32, tag="g", name="g")
                    # globals first on the PE and first on ACT: the chain
                    # glob-scores -> exp-G -> gmask-mul gates the next slot's
                    # sums/PV start (the full-width start=True matmul), so it
                    # gets maximum slack; window tiles follow.
                    nc.tensor.matmul(
                        gg[:], kT_sb[:, g, KWIN:KWIN + 128],
                        qT_all[:, h, :], start=True, stop=True,
                    )
                    pT = pTp.tile([128, 4, TQ], bf, tag="pT", name=f"pT{h % 3}")
                    pT_tiles[h] = pT
                    nc.scalar.activation(pT[:, 3, :], gg[:], EXP, scale=SCALE)
                    pstile = {0: ab, 1: ab, 2: cc}
                    off = {0: 0, 1: 512, 2: 0}
                    for kw, slot, col0, q0, q1 in KW_LAYOUT:
                        ps = pstile[slot]
                        c0 = off[slot] + col0
                        nc.tensor.matmul(
                            ps[:, c0:c0 + (q1 - q0)],
                            kT_sb[:, g, kw * 128:(kw + 1) * 128],
                            qT_all[:, h, q0:q1],
                            start=True, stop=True,
                        )
                    nc.scalar.activation(
                        pT[:, 0:2, :], ab[:].rearrange("p (a b) -> p a b", a=2),
                        EXP, scale=SCALE)
                    nc.scalar.activation(pT[:, 2, :], cc[:], EXP, scale=SCALE)

                def mask(h):
                    pT = pT_tiles[h]
                    # gmask first: it gates the sums/PV start matmul
                    nc.vector.tensor_mul(pT[:, 3, :], pT[:, 3, :], gmask_sb[:])
                    for slot, col0, _l in DIAG_SLICES:
                        nc.vector.tensor_mul(
                            pT[:, slot, col0:col0 + 128],
                            pT[:, slot, col0:col0 + 128], diag_sb[:])

                def sums_pv(h):
                    g = h // 4
                    pT = pT_tiles[h]
                    ss = psS.tile([128, TQ], f32, tag="s", name="s")
                    yy = psY.tile([128, TQ], f32, tag="y", name="y")
                    # globals first: full-width start initializes every column.
                    # The ones-matrix stationary writes the column sums to all
                    # 128 partitions, i.e. the softmax denominators arrive
                    # pre-broadcast for the y^T normalize.
                    nc.tensor.matmul(ss[:], wones_sb[:, 6, :], pT[:, 3, :],
                                     start=True, stop=False, skip_group_check=True)
                    nc.tensor.matmul(yy[:], V_sb[:, 6, g * HD:(g + 1) * HD],
                                     pT[:, 3, :],
                                     start=True, stop=False, skip_group_check=True)
                    for i, (kw, slot, col0, q0, q1) in enumerate(KW_LAYOUT):
                        last = i == len(KW_LAYOUT) - 1
                        nc.tensor.matmul(
                            ss[:, q0:q1], wones_sb[:, kw, :],
                            pT[:, slot, col0:col0 + (q1 - q0)],
                            start=False, stop=last, skip_group_check=True)
                        nc.tensor.matmul(
                            yy[:, q0:q1], V_sb[:, kw, g * HD:(g + 1) * HD],
                            pT[:, slot, col0:col0 + (q1 - q0)],
                            start=False, stop=last, skip_group_check=True)
                    return ss, yy

                def norm(h, ss, yy):
                    recip = nrm.tile([128, TQ], f32, tag="recip")
                    nc.vector.reciprocal_approx_fast(out=recip[:], in_=ss[:])
                    nc.vector.tensor_mul(yT_sb[:, h, :], yy[:], recip[:])

                for h in range(NH):
                    scores(h)
                    if h > 0:
                        ss, yy = sums_pv(h - 1)
                        norm(h - 1, ss, yy)
                    mask(h)
                ss, yy = sums_pv(NH - 1)
                norm(NH - 1, ss, yy)

            # ---- Stage D: out^T = Wo^T @ y^T ----
            with tc.tile_pool(name="tmpD", bufs=3) as tmpD, \
                 tc.tile_pool(name="psD", bufs=2, space="PSUM") as psD:
                for od in range(16):
                    ps_o = psD.tile([128, TQ], f32, tag="o")
                    if od < 15:
                        for hk in range(16):
                            nc.tensor.matmul(
                                ps_o[:], wo_all[:, od, hk, :], yT_sb[:, hk, :],
                                start=(hk == 0), stop=(hk == 15),
                            )
                        ob = tmpD.tile([128, TQ], f32, tag="ob")
                        nc.any.tensor_copy(ob[:], ps_o[:])
                        nc.sync.dma_start(outT[od * 128:(od + 1) * 128, :],
                                          ob[:])
                    else:
                        # split the last chunk so copy+store of the first half
                        # overlap the second half's matmuls (shorter tail)
                        ob = tmpD.tile([128, TQ], f32, tag="ob")
                        for q0, q1 in ((0, 256), (256, 512)):
                            for hk in range(16):
                                nc.tensor.matmul(
                                    ps_o[:, q0:q1], wo_all[:, od, hk, :],
                                    yT_sb[:, hk, q0:q1],
                                    start=(hk == 0), stop=(hk == 15),
                                )
                            nc.any.tensor_copy(ob[:, q0:q1], ps_o[:, q0:q1])
                            nc.sync.dma_start(
                                outT[od * 128:(od + 1) * 128, q0:q1],
                                ob[:, q0:q1])

    nc.finalize()
    return nc


@functools.lru_cache(maxsize=1)
def _program():
    return _build_program()


def _rope_tables():
    freqs = 1.0 / (THETA ** (np.arange(0, HD, 2, dtype=np.float32) / HD))
    emb = np.arange(T, dtype=np.float32)[:, None] * freqs[None, :]  # [T, 64]
    cos = np.concatenate([np.cos(emb), np.cos(emb)], axis=-1)  # [T, 128]
    sin = np.concatenate([np.sin(emb), np.sin(emb)], axis=-1)
    return cos.astype(np.float32), sin.astype(np.float32)


def _masked(qpos, kpos):
    """Reference sparsity rule. qpos [Q], kpos [K] -> bool [Q, K] (True=masked)."""
    qb = qpos[:, None] // BLOCK
    kb = kpos[None, :] // BLOCK
    future = kb > qb
    outside = np.abs(kpos[None, :] - qpos[:, None]) > WINDOW
    glob = (kpos[None, :] % GEV) == 0
    return (outside & ~glob) | future


def _core_inputs(x, Wq, Wkv, Wku, Wvu, Wo, cos, sin, b, ch):
    t0 = ch * TQ
    kp = np.full(KT, -1, dtype=np.int64)
    kp[0:KWIN] = np.arange(t0 - WINDOW, t0 + TQ)
    # global slots: every global token below the l=3 window floor
    globpos = np.arange(0, max(0, t0 - WINDOW + 3 * 128), GEV)
    assert len(globpos) <= NG
    kp[KWIN:KWIN + len(globpos)] = globpos
    valid = kp >= 0

    xT = np.zeros((D, KT), BF16)
    xT[:, valid] = x[b, kp[valid]].T.astype(BF16)
    cosT = np.zeros((HD, KT), np.float32)
    sinT = np.zeros((HD, KT), np.float32)
    cosT[:, valid] = cos[kp[valid]].T
    sinT[:, valid] = sin[kp[valid]].T

    # globals 0/1 keep-mask [128 glob rows, 512 q]: row j = glob slot j
    # (rows >= 32 are kT pad -> 0), col q: visible iff slot valid, not a
    # future block, and below the window floor for q's l-block.
    gmask = np.zeros((128, TQ), np.float32)
    qpos = t0 + np.arange(TQ)
    for j, kg in enumerate(kp[KWIN:KWIN + NG]):
        if kg < 0:
            continue
        vis = ~_masked(qpos, np.array([kg]))[:, 0]
        l = np.arange(TQ) // 128
        vis &= kg < (t0 - WINDOW + l * 128)  # else counted in window tiles
        gmask[j, :] = vis.astype(np.float32)

    # per-kw ones matrices for the sums matmul (the [128,128] stationary
    # broadcasts the column sums to every output partition); kw tiles that
    # are entirely padding (chunk 0 edge) contribute exp(0)=1 -> zero them.
    wones = np.ones((128, 7, 128), np.float32)
    for kw in range(6):
        if kp[kw * 128] < 0 and kp[(kw + 1) * 128 - 1] < 0:
            wones[:, kw, :] = 0.0

    # diag keep-mask [k_in_tile, q_in_block]: visible iff q <= k or key is
    # global (key block is exactly 2 before query block in diag tiles)
    kk = np.arange(128)[:, None]
    ii = np.arange(128)[None, :]
    diag01 = ((ii <= kk) | (kk % GEV == 0)).astype(np.float32)

    rotM = np.zeros((HD, HD), np.float32)
    rotM[np.arange(64), np.arange(64) + 64] = 1.0   # RT[a, a+64] = +1 (a < 64)
    rotM[np.arange(64) + 64, np.arange(64)] = -1.0  # RT[a, a-64] = -1 (a >= 64)

    wqT = np.ascontiguousarray(
        Wq.reshape(16, 128, NH, HD).transpose(2, 1, 0, 3)).astype(BF16)
    woT = np.ascontiguousarray(
        Wo.reshape(16, 128, 16, HD).transpose(2, 1, 0, 3)).astype(BF16)
    return dict(xT=xT,
                wq=wqT, wkv=Wkv.astype(BF16),
                wku=Wku.astype(BF16), wvu=Wvu.astype(BF16),
                wo=woT,
                cosT=cosT, sinT=sinT,
                gmask=gmask.astype(BF16), wones=wones.astype(BF16),
                diag01=diag01.astype(BF16),
                rotM=rotM.astype(BF16))


def _run(in_maps, trace=False):
    from concourse.bass_utils import run_bass_kernel_spmd
    nc = _program()
    kwargs = {}
    if trace:
        kwargs = dict(trace=True, trace_cores=list(range(NCORES)))
    return run_bass_kernel_spmd(nc, in_maps, core_ids=list(range(NCORES)),
                                **kwargs)


def kernel(x, Wq, Wkv_down, Wk_up, Wv_up, Wo, _trace=False):
    x = np.ascontiguousarray(np.asarray(x, dtype=np.float32))
    Wq = np.ascontiguousarray(np.asarray(Wq, dtype=np.float32))
    Wkv_down = np.ascontiguousarray(np.asarray(Wkv_down, dtype=np.float32))
    Wk_up = np.ascontiguousarray(np.asarray(Wk_up, dtype=np.float32))
    Wv_up = np.ascontiguousarray(np.asarray(Wv_up, dtype=np.float32))
    Wo = np.ascontiguousarray(np.asarray(Wo, dtype=np.float32))

    cos, sin = _rope_tables()
    in_maps = []
    for c in range(NCORES):
        b, ch = divmod(c, 4)
        in_maps.append(_core_inputs(x, Wq, Wkv_down, Wk_up, Wv_up, Wo,
                                    cos, sin, b, ch))
    res = _run(in_maps, trace=_trace)
    out = np.empty((B, T, D), np.float32)
    for c in range(NCORES):
        b, ch = divmod(c, 4)
        out[b, ch * TQ:(ch + 1) * TQ, :] = res.results[c]["outT"].T
    if _trace:
        kernel.last_results = res
    return out


# revision 42
# speedup vs baseline: 1.1536x; 1.1536x over previous
"""Block-causal sparse attention (MLA latent KV + GQA + RoPE) on 8 TRN2 cores.

Sharding: 2 batches x 4 query-chunks of 512 tokens (T-sharding). Each core
computes its 512 output rows end-to-end (q/kv projections, sparse attention,
o-projection) over a gathered key set of 768 window rows + 32 global slots.
Everything runs in a transposed layout (feature dim on partitions): the host
supplies x^T slices and the kernel returns out^T.

Attention stage: scores are computed directly in S^T layout (keys on
partitions, queries on free dim), so P^T = exp(S^T) feeds the P@V matmul
with NO PE transposes and no per-(head,l) DVE normalize/copy chains.
Masking is a 0/1 multiply after exp (strict-lower-triangle diag mask +
per-chunk globals mask). Softmax sums come from a [128,128] ones-matrix
stationary matmul over P^T — the output arrives pre-broadcast on all 128
partitions, so normalization is one reciprocal_approx_fast + one DVE
multiply on y^T per head with no cross-engine broadcast (avoids a
PE->DVE->GpSimd->ACT semaphore convoy).

Other perf-relevant choices (each validated against a perfetto trace):
- all matmul operands bf16 (fp32 PSUM accumulation)
- x^T resident in SBUF; stage A (kv-down) streams it chunk-by-chunk and
  C1 (q-proj) reads it in place — no DVE staging copies
- Wq/Wo are pre-transposed on the host into per-chunk-contiguous blocks:
  a strided on-the-fly rearrange DMA costs ~4.2us/chunk (256B elements)
  vs ~0.6us contiguous, and the stalls re-throttle the PE clock (HAM)
- DMA priority order on one HWDGE queue: x/wkv, first 3 wq chunks, rope
  tables, wku/wvu + masks, remaining wq (pool-paced), all of Wo
  (prefetched during B/C2), outputs
- C1 and C2 are software-pipelined one head deep; stage B interleaves
  V-tile matmuls into the K rope chain to keep the PE dense
- PSUM is budgeted to exactly 8 banks in C2 (AB x2, C, G, y, sums)
"""

import functools
import numpy as np
import ml_dtypes

BF16 = ml_dtypes.bfloat16

# Model constants (hardcoded per problem spec)
D = 2048        # d_model
L = 512         # MLA latent
NH = 16         # query heads
NKV = 4         # kv heads
HD = 128        # head dim
B, T = 2, 2048
BLOCK = 128
WINDOW = 256
GEV = 64        # global every
THETA = 10000.0

# Sharding geometry
NCORES = 8
TQ = 512        # queries per core
KWIN = 768      # window key rows (t0-256 .. t0+512)
NG = 32         # global slots (padded)
KT = 896        # key layout: 768 window | 32 global | 96 zero pad
KQ0 = 256       # query cols inside key layout
SCALE = 1.0 / float(np.sqrt(HD))

# S^T PSUM packing: (kw, psum tag, col offset, q range)
#   tag AB = [128,1024]: kw0@0 (q0:128) | kw2@128 (q0:384) | kw1@512 (q0:256)
#                        | kw4@768 (q256:512)
#   tag C  = [128,512]:  kw3@0 (q128:512) | kw5@384 (q384:512)
#   tag G  = [128,512]:  globals (q0:512)
# pT SBUF mirror: [128, 4, 512] bf16; slot0/1 = AB, slot2 = C, slot3 = G.
KW_LAYOUT = [
    # kw, slot, col0, q0, q1
    (0, 0, 0,   0,   128),
    (2, 0, 128, 0,   384),
    (1, 1, 0,   0,   256),
    (4, 1, 256, 256, 512),
    (3, 2, 0,   128, 512),
    (5, 2, 384, 384, 512),
]
# diag tiles (kw == l): (slot, col offset in slot, l)
DIAG_SLICES = [(0, 0, 0), (1, 128, 1), (0, 384, 2), (2, 256, 3)]


def _build_program(loop_n=None):
    import contextlib
    import concourse.bacc as bacc
    import concourse.tile as tile
    import concourse.mybir as mybir

    f32 = mybir.dt.float32
    bf = mybir.dt.bfloat16
    EXP = mybir.ActivationFunctionType.Exp
    CPY = mybir.ActivationFunctionType.Copy

    nc = bacc.Bacc("TRN2", target_bir_lowering=False, debug=False)

    xT = nc.dram_tensor("xT", [D, KT], bf, kind="ExternalInput")
    wq = nc.dram_tensor("wq", [NH, 128, 16, HD], bf, kind="ExternalInput")
    wkv = nc.dram_tensor("wkv", [D, L], bf, kind="ExternalInput")
    wku = nc.dram_tensor("wku", [L, NKV * HD], bf, kind="ExternalInput")
    wvu = nc.dram_tensor("wvu", [L, NKV * HD], bf, kind="ExternalInput")
    wo = nc.dram_tensor("wo", [16, 128, 16, HD], bf, kind="ExternalInput")
    cosT = nc.dram_tensor("cosT", [HD, KT], f32, kind="ExternalInput")
    sinT = nc.dram_tensor("sinT", [HD, KT], f32, kind="ExternalInput")
    rotM = nc.dram_tensor("rotM", [HD, HD], bf, kind="ExternalInput")
    gmask = nc.dram_tensor("gmask", [128, TQ], bf, kind="ExternalInput")
    wones = nc.dram_tensor("wones", [128, 7, 128], bf, kind="ExternalInput")
    diag01 = nc.dram_tensor("diag01", [128, 128], bf, kind="ExternalInput")
    outT = nc.dram_tensor("outT", [D, TQ], f32, kind="ExternalOutput")

    def act_copy(out, in_):
        nc.scalar.activation(out, in_, CPY)

    with tile.TileContext(nc) as tc, contextlib.ExitStack() as _es:
        if loop_n:
            _es.enter_context(tc.For_i(0, loop_n, 1))
        with tc.tile_pool(name="const", bufs=1) as constp:
            qT_all = constp.tile([HD, NH, TQ], bf)       # roped q^T per head
            warm = constp.tile([1, 2], f32)
            nc.vector.memset(warm[:], 0.0)
            act_warm = constp.tile([1, 2], f32)
            nc.scalar.activation(act_warm[:], warm[:],
                                 mybir.ActivationFunctionType.Exp)
            kT_sb = constp.tile([HD, NKV, KT], bf)       # roped K^T per kv head
            V_sb = constp.tile([128, 7, NKV * HD], bf)   # V rows x (kv*hd)
            yT_sb = constp.tile([HD, NH, TQ], bf)        # attention out^T
            wo_all = constp.tile([128, 16, 16, HD], bf)  # all of Wo^T, chunked

            with tc.tile_pool(name="ckvp", bufs=1) as ckvp:
                ckv_sb = ckvp.tile([128, 4, KT], bf)
                wku_sb = ckvp.tile([128, 4, NKV * HD], bf)
                wvu_sb = ckvp.tile([128, 4, NKV * HD], bf)

                with tc.tile_pool(name="xp", bufs=1) as xp, \
                     tc.tile_pool(name="wkvp", bufs=8) as wkvp:
                    xfull = xp.tile([128, 16, KT], bf)   # x^T resident
                    # x^T/wkv DMAs first — stage A blocks on chunk 0 and
                    # consumes chunk k as it lands; everything else queues
                    # behind in priority order (see module docstring).
                    wkv_k = [None] * 16
                    for k in range(16):
                        wkv_k[k] = wkvp.tile([128, L], bf, tag="wkv",
                                             name=f"wkv{k % 8}")
                        if k == 0:
                            # first matmul blocks on wkv0 + x0's first half:
                            # load those first, in the smallest pieces
                            nc.sync.dma_start(wkv_k[0][:], wkv[0:128, :])
                            nc.sync.dma_start(xfull[:, 0, 0:512],
                                              xT[0:128, 0:512])
                            nc.sync.dma_start(xfull[:, 0, 512:KT],
                                              xT[0:128, 512:KT])
                        else:
                            nc.sync.dma_start(xfull[:, k, :],
                                              xT[k * 128:(k + 1) * 128, :])
                            nc.sync.dma_start(wkv_k[k][:],
                                              wkv[k * 128:(k + 1) * 128, :])

                    # ---- Stage A: c_kv^T = Wkv_down^T @ x^T -> [L, KT] ----
                    with tc.tile_pool(name="psA", bufs=1, space="PSUM") as psA:
                        ps_ckv = [psA.tile([128, KT], f32, tag=f"ckv{lt}",
                                           name=f"ckv{lt}")
                                  for lt in range(4)]
                        for k in range(16):
                            for lt in range(4):
                                for c0, c1 in ((0, 512), (512, 800)):
                                    nc.tensor.matmul(
                                        ps_ckv[lt][:, c0:c1],
                                        wkv_k[k][:, lt * 128:(lt + 1) * 128],
                                        xfull[:, k, c0:c1],
                                        start=(k == 0), stop=(k == 15),
                                    )
                        # cols 800:896 are key-layout padding: zero instead of
                        # computing (downstream K/V then see exact zeros)
                        for lt in range(4):
                            if lt % 2 == 0:
                                nc.vector.tensor_copy(ckv_sb[:, lt, 0:800],
                                                      ps_ckv[lt][:, 0:800])
                            else:
                                act_copy(ckv_sb[:, lt, 0:800],
                                         ps_ckv[lt][:, 0:800])
                            nc.vector.memset(ckv_sb[:, lt, 800:KT], 0.0)

                    # ---- Stage C1: q projection + RoPE for all heads ----
                    # DMA priority order: first 3 wq chunks right behind x
                    # (C1's first matmuls), then the small rope constants,
                    # then the remaining wq chunks paced by the pool.
                    with tc.tile_pool(name="wqp", bufs=4) as wqp, \
                         tc.tile_pool(name="tmpQ", bufs=2) as tmpQ, \
                         tc.tile_pool(name="psq", bufs=2, space="PSUM") as psq, \
                         tc.tile_pool(name="psr", bufs=2, space="PSUM") as psr:
                        wq_pre = []
                        for h in range(3):
                            wq_t = wqp.tile([128, 16, HD], bf, tag="wqh",
                                            name=f"wq{h % 4}")
                            nc.sync.dma_start(wq_t[:], wq[h])
                            wq_pre.append(wq_t)
                        cos_sb = constp.tile([HD, KT], f32)
                        nc.sync.dma_start(cos_sb[:], cosT[:])
                        sin_sb = constp.tile([HD, KT], f32)
                        nc.sync.dma_start(sin_sb[:], sinT[:])
                        rot_sb = constp.tile([HD, HD], bf)
                        nc.sync.dma_start(rot_sb[:], rotM[:])
                        for lk in range(4):
                            nc.sync.dma_start(wku_sb[:, lk, :],
                                              wku[lk * 128:(lk + 1) * 128, :])
                            nc.sync.dma_start(wvu_sb[:, lk, :],
                                              wvu[lk * 128:(lk + 1) * 128, :])
                        gmask_sb = constp.tile([128, TQ], bf)
                        nc.sync.dma_start(gmask_sb[:], gmask[:])
                        wones_sb = constp.tile([128, 7, 128], bf)
                        nc.sync.dma_start(wones_sb[:], wones[:])
                        diag_sb = constp.tile([128, 128], bf)
                        nc.sync.dma_start(diag_sb[:], diag01[:])
                        # software pipeline: head h's rope rotation matmul
                        # is emitted after head h+1's projection matmuls so the
                        # PE never waits on the ACT psum->bf16 copy
                        pend = [None] * NH

                        def q_proj(h, wq_h):
                            ps_q = psq.tile([128, TQ], f32, tag="q",
                                            name=f"q{h % 2}")
                            for k in range(16):
                                nc.tensor.matmul(
                                    ps_q[:], wq_h[:, k, :],
                                    xfull[:, k, KQ0:KQ0 + TQ],
                                    start=(k == 0), stop=(k == 15),
                                )
                            qh_r = tmpQ.tile([128, TQ], bf, tag="qhr",
                                             name=f"qhr{h % 2}")
                            act_copy(qh_r[:], ps_q[:])
                            t_qc = tmpQ.tile([128, TQ], f32, tag="tqc",
                                             name=f"tqc{h % 2}")
                            nc.vector.tensor_mul(t_qc[:], ps_q[:],
                                                 cos_sb[:, KQ0:KQ0 + TQ])
                            pend[h] = (qh_r, t_qc)

                        def q_rope(h):
                            qh_r, t_qc = pend[h]
                            ps_rt = psr.tile([128, TQ], f32, tag="qrot")
                            nc.tensor.matmul(ps_rt[:], rot_sb[:], qh_r[:],
                                             start=True, stop=True)
                            qt1 = tmpQ.tile([128, TQ], f32, tag="qt1")
                            nc.vector.tensor_mul(qt1[:], ps_rt[:],
                                                 sin_sb[:, KQ0:KQ0 + TQ])
                            nc.vector.tensor_add(qT_all[:, h, :], t_qc[:],
                                                 qt1[:])

                        for h in range(NH):
                            if h < 3:
                                wq_h = wq_pre[h]
                            else:
                                wq_h = wqp.tile([128, 16, HD], bf, tag="wqh",
                                                name=f"wq{h % 4}")
                                nc.sync.dma_start(wq_h[:], wq[h])
                            q_proj(h, wq_h)
                            if h > 0:
                                q_rope(h - 1)
                        q_rope(NH - 1)

                # Wo prefetch: queued on sync behind wq, runs during B/C2.
                for od in range(16):
                    nc.sync.dma_start(wo_all[:, od, :, :], wo[od])

                # ---- Stage B: K^T (roped) and V, interleaved for overlap ----
                with tc.tile_pool(name="tmpB", bufs=2) as tmpB, \
                     tc.tile_pool(name="pskh", bufs=2, space="PSUM") as pskh, \
                     tc.tile_pool(name="psrot", bufs=1, space="PSUM") as psrot, \
                     tc.tile_pool(name="psv", bufs=2, space="PSUM") as psv:
                    def k_head(g):
                        ps_kh = pskh.tile([128, KT], f32, tag="kh")
                        for lk in range(4):
                            for c0, c1 in ((0, 512), (512, 800)):
                                nc.tensor.matmul(
                                    ps_kh[:, c0:c1],
                                    wku_sb[:, lk, g * 128:(g + 1) * 128],
                                    ckv_sb[:, lk, c0:c1],
                                    start=(lk == 0), stop=(lk == 3),
                                )
                        return ps_kh

                    def k_rope(g, ps_kh):
                        # only the 800 real key columns; kT pad is memset so
                        # the globals-tile stationary reads exact zeros
                        kh_r = tmpB.tile([128, KT], bf, tag="khr")
                        act_copy(kh_r[:, 0:800], ps_kh[:, 0:800])
                        t_kc = tmpB.tile([128, KT], f32, tag="tkc")
                        nc.vector.tensor_mul(t_kc[:, 0:800], ps_kh[:, 0:800],
                                             cos_sb[:, 0:800])
                        ps_rot = psrot.tile([128, KT], f32, tag="rot")
                        for c0, c1 in ((0, 512), (512, 800)):
                            nc.tensor.matmul(ps_rot[:, c0:c1], rot_sb[:],
                                             kh_r[:, c0:c1], start=True, stop=True)
                        t1 = tmpB.tile([128, KT], f32, tag="t1")
                        nc.vector.tensor_mul(t1[:, 0:800], ps_rot[:, 0:800],
                                             sin_sb[:, 0:800])
                        nc.vector.tensor_add(kT_sb[:, g, 0:800], t_kc[:, 0:800],
                                             t1[:, 0:800])
                        nc.vector.memset(kT_sb[:, g, 800:KT], 0.0)

                    def v_tile(tt):
                        ps_v = psv.tile([128, 512], f32, tag="v")
                        for lk in range(4):
                            nc.tensor.matmul(
                                ps_v[:],
                                ckv_sb[:, lk, tt * 128:(tt + 1) * 128],
                                wvu_sb[:, lk, :],
                                start=(lk == 0), stop=(lk == 3),
                            )
                        if tt % 2 == 0:
                            nc.vector.tensor_copy(V_sb[:, tt, :], ps_v[:])
                        else:
                            act_copy(V_sb[:, tt, :], ps_v[:])

                    # interleave: V matmuls keep the PE busy while the DVE
                    # rope chain for each kv head drains
                    ps = k_head(0); v_tile(0)
                    k_rope(0, ps); v_tile(1)
                    ps = k_head(1); v_tile(2)
                    k_rope(1, ps); v_tile(3)
                    ps = k_head(2); v_tile(4)
                    k_rope(2, ps)
                    ps = k_head(3); v_tile(5)
                    k_rope(3, ps); v_tile(6)

            # ---- Stage C2: sparse attention in S^T layout, software-
            # pipelined over heads (PV/sums/norm of head h-1 run between
            # head h's scores and its exp/mask chain) ----
            with tc.tile_pool(name="pTp", bufs=3) as pTp, \
                 tc.tile_pool(name="nrm", bufs=2) as nrm, \
                 tc.tile_pool(name="psAB", bufs=2, space="PSUM") as psAB, \
                 tc.tile_pool(name="psC", bufs=1, space="PSUM") as psC, \
                 tc.tile_pool(name="psG", bufs=1, space="PSUM") as psG, \
                 tc.tile_pool(name="psY", bufs=1, space="PSUM") as psY, \
                 tc.tile_pool(name="psS", bufs=1, space="PSUM") as psS:

                pT_tiles = [None] * NH

                def scores(h):
                    g = h // 4
                    ab = psAB.tile([128, 1024], f32, tag="ab", name="ab")
                    cc = psC.tile([128, 512], f32, tag="c", name="c")
                    gg = psG.tile([128, 512], f32, tag="g", name="g")
                    pstile = {0: ab, 1: ab, 2: cc}
                    off = {0: 0, 1: 512, 2: 0}
                    for kw, slot, col0, q0, q1 in KW_LAYOUT:
                        ps = pstile[slot]
                        c0 = off[slot] + col0
                        nc.tensor.matmul(
                            ps[:, c0:c0 + (q1 - q0)],
                            kT_sb[:, g, kw * 128:(kw + 1) * 128],
                            qT_all[:, h, q0:q1],
                            start=True, stop=True,
                        )
                    nc.tensor.matmul(
                        gg[:], kT_sb[:, g, KWIN:KWIN + 128],
                        qT_all[:, h, :], start=True, stop=True,
                    )
                    # exp (scaled); P^T lands in SBUF in the psum packing
                    pT = pTp.tile([128, 4, TQ], bf, tag="pT", name=f"pT{h % 3}")
                    pT_tiles[h] = pT
                    nc.scalar.activation(
                        pT[:, 0:2, :], ab[:].rearrange("p (a b) -> p a b", a=2),
                        EXP, scale=SCALE)
                    nc.scalar.activation(pT[:, 2, :], cc[:], EXP, scale=SCALE)
                    nc.scalar.activation(pT[:, 3, :], gg[:], EXP, scale=SCALE)

                def mask(h):
                    pT = pT_tiles[h]
                    for slot, col0, _l in DIAG_SLICES:
                        nc.vector.tensor_mul(
                            pT[:, slot, col0:col0 + 128],
                            pT[:, slot, col0:col0 + 128], diag_sb[:])
                    nc.vector.tensor_mul(pT[:, 3, :], pT[:, 3, :], gmask_sb[:])

                def sums_pv(h):
                    g = h // 4
                    pT = pT_tiles[h]
                    ss = psS.tile([128, TQ], f32, tag="s", name="s")
                    yy = psY.tile([128, TQ], f32, tag="y", name="y")
                    # globals first: full-width start initializes every column.
                    # The ones-matrix stationary writes the column sums to all
                    # 128 partitions, i.e. the softmax denominators arrive
                    # pre-broadcast for the y^T normalize.
                    nc.tensor.matmul(ss[:], wones_sb[:, 6, :], pT[:, 3, :],
                                     start=True, stop=False, skip_group_check=True)
                    nc.tensor.matmul(yy[:], V_sb[:, 6, g * HD:(g + 1) * HD],
                                     pT[:, 3, :],
                                     start=True, stop=False, skip_group_check=True)
                    for i, (kw, slot, col0, q0, q1) in enumerate(KW_LAYOUT):
                        last = i == len(KW_LAYOUT) - 1
                        nc.tensor.matmul(
                            ss[:, q0:q1], wones_sb[:, kw, :],
                            pT[:, slot, col0:col0 + (q1 - q0)],
                            start=False, stop=last, skip_group_check=True)
                        nc.tensor.matmul(
                            yy[:, q0:q1], V_sb[:, kw, g * HD:(g + 1) * HD],
                            pT[:, slot, col0:col0 + (q1 - q0)],
                            start=False, stop=last, skip_group_check=True)
                    return ss, yy

                def norm(h, ss, yy):
                    recip = nrm.tile([128, TQ], f32, tag="recip")
                    nc.vector.reciprocal_approx_fast(out=recip[:], in_=ss[:])
                    nc.vector.tensor_mul(yT_sb[:, h, :], yy[:], recip[:])

                for h in range(NH):
                    scores(h)
                    if h > 0:
                        ss, yy = sums_pv(h - 1)
                        norm(h - 1, ss, yy)
                    mask(h)
                ss, yy = sums_pv(NH - 1)
                norm(NH - 1, ss, yy)

            # ---- Stage D: out^T = Wo^T @ y^T ----
            with tc.tile_pool(name="tmpD", bufs=3) as tmpD, \
                 tc.tile_pool(name="psD", bufs=2, space="PSUM") as psD:
                for od in range(16):
                    ps_o = psD.tile([128, TQ], f32, tag="o")
                    if od < 15:
                        for hk in range(16):
                            nc.tensor.matmul(
                                ps_o[:], wo_all[:, od, hk, :], yT_sb[:, hk, :],
                                start=(hk == 0), stop=(hk == 15),
                            )
                        ob = tmpD.tile([128, TQ], f32, tag="ob")
                        nc.any.tensor_copy(ob[:], ps_o[:])
                        nc.sync.dma_start(outT[od * 128:(od + 1) * 128, :],
                                          ob[:])
                    else:
                        # split the last chunk so copy+store of the first half
                        # overlap the second half's matmuls (shorter tail)
                        ob = tmpD.tile([128, TQ], f32, tag="ob")
                        for q0, q1 in ((0, 256), (256, 512)):
                            for hk in range(16):
                                nc.tensor.matmul(
                                    ps_o[:, q0:q1], wo_all[:, od, hk, :],
                                    yT_sb[:, hk, q0:q1],
                                    start=(hk == 0), stop=(hk == 15),
                                )
                            nc.any.tensor_copy(ob[:, q0:q1], ps_o[:, q0:q1])
                            nc.sync.dma_start(
                                outT[od * 128:(od + 1) * 128, q0:q1],
                                ob[:, q0:q1])

    nc.finalize()
    return nc


@functools.lru_cache(maxsize=1)
def _program():
    return _build_program()


def _rope_tables():
    freqs = 1.0 / (THETA ** (np.arange(0, HD, 2, dtype=np.float32) / HD))
    emb = np.arange(T, dtype=np.float32)[:, None] * freqs[None, :]  # [T, 64]
    cos = np.concatenate([np.cos(emb), np.cos(emb)], axis=-1)  # [T, 128]
    sin = np.concatenate([np.sin(emb), np.sin(emb)], axis=-1)
    return cos.astype(np.float32), sin.astype(np.float32)


def _masked(qpos, kpos):
    """Reference sparsity rule. qpos [Q], kpos [K] -> bool [Q, K] (True=masked)."""
    qb = qpos[:, None] // BLOCK
    kb = kpos[None, :] // BLOCK
    future = kb > qb
    outside = np.abs(kpos[None, :] - qpos[:, None]) > WINDOW
    glob = (kpos[None, :] % GEV) == 0
    return (outside & ~glob) | future


def _core_inputs(x, Wq, Wkv, Wku, Wvu, Wo, cos, sin, b, ch):
    t0 = ch * TQ
    kp = np.full(KT, -1, dtype=np.int64)
    kp[0:KWIN] = np.arange(t0 - WINDOW, t0 + TQ)
    # global slots: every global token below the l=3 window floor
    globpos = np.arange(0, max(0, t0 - WINDOW + 3 * 128), GEV)
    assert len(globpos) <= NG
    kp[KWIN:KWIN + len(globpos)] = globpos
    valid = kp >= 0

    xT = np.zeros((D, KT), BF16)
    xT[:, valid] = x[b, kp[valid]].T.astype(BF16)
    cosT = np.zeros((HD, KT), np.float32)
    sinT = np.zeros((HD, KT), np.float32)
    cosT[:, valid] = cos[kp[valid]].T
    sinT[:, valid] = sin[kp[valid]].T

    # globals 0/1 keep-mask [128 glob rows, 512 q]: row j = glob slot j
    # (rows >= 32 are kT pad -> 0), col q: visible iff slot valid, not a
    # future block, and below the window floor for q's l-block.
    gmask = np.zeros((128, TQ), np.float32)
    qpos = t0 + np.arange(TQ)
    for j, kg in enumerate(kp[KWIN:KWIN + NG]):
        if kg < 0:
            continue
        vis = ~_masked(qpos, np.array([kg]))[:, 0]
        l = np.arange(TQ) // 128
        vis &= kg < (t0 - WINDOW + l * 128)  # else counted in window tiles
        gmask[j, :] = vis.astype(np.float32)

    # per-kw ones matrices for the sums matmul (the [128,128] stationary
    # broadcasts the column sums to every output partition); kw tiles that
    # are entirely padding (chunk 0 edge) contribute exp(0)=1 -> zero them.
    wones = np.ones((128, 7, 128), np.float32)
    for kw in range(6):
        if kp[kw * 128] < 0 and kp[(kw + 1) * 128 - 1] < 0:
            wones[:, kw, :] = 0.0

    # diag keep-mask [k_in_tile, q_in_block]: visible iff q <= k or key is
    # global (key block is exactly 2 before query block in diag tiles)
    kk = np.arange(128)[:, None]
    ii = np.arange(128)[None, :]
    diag01 = ((ii <= kk) | (kk % GEV == 0)).astype(np.float32)

    rotM = np.zeros((HD, HD), np.float32)
    rotM[np.arange(64), np.arange(64) + 64] = 1.0   # RT[a, a+64] = +1 (a < 64)
    rotM[np.arange(64) + 64, np.arange(64)] = -1.0  # RT[a, a-64] = -1 (a >= 64)

    wqT = np.ascontiguousarray(
        Wq.reshape(16, 128, NH, HD).transpose(2, 1, 0, 3)).astype(BF16)
    woT = np.ascontiguousarray(
        Wo.reshape(16, 128, 16, HD).transpose(2, 1, 0, 3)).astype(BF16)
    return dict(xT=xT,
                wq=wqT, wkv=Wkv.astype(BF16),
                wku=Wku.astype(BF16), wvu=Wvu.astype(BF16),
                wo=woT,
                cosT=cosT, sinT=sinT,
                gmask=gmask.astype(BF16), wones=wones.astype(BF16),
                diag01=diag01.astype(BF16),
                rotM=rotM.astype(BF16))


def _run(in_maps, trace=False):
    from concourse.bass_utils import run_bass_kernel_spmd
    nc = _program()
    kwargs = {}
    if trace:
        kwargs = dict(trace=True, trace_cores=list(range(NCORES)))
    return run_bass_kernel_spmd(nc, in_maps, core_ids=list(range(NCORES)),
                                **kwargs)


def kernel(x, Wq, Wkv_down, Wk_up, Wv_up, Wo, _trace=False):
    x = np.ascontiguousarray(np.asarray(x, dtype=np.float32))
    Wq = np.ascontiguousarray(np.asarray(Wq, dtype=np.float32))
    Wkv_down = np.ascontiguousarray(np.asarray(Wkv_down, dtype=np.float32))
    Wk_up = np.ascontiguousarray(np.asarray(Wk_up, dtype=np.float32))
    Wv_up = np.ascontiguousarray(np.asarray(Wv_up, dtype=np.float32))
    Wo = np.ascontiguousarray(np.asarray(Wo, dtype=np.float32))

    cos, sin = _rope_tables()
    in_maps = []
    for c in range(NCORES):
        b, ch = divmod(c, 4)
        in_maps.append(_core_inputs(x, Wq, Wkv_down, Wk_up, Wv_up, Wo,
                                    cos, sin, b, ch))
    res = _run(in_maps, trace=_trace)
    out = np.empty((B, T, D), np.float32)
    for c in range(NCORES):
        b, ch = divmod(c, 4)
        out[b, ch * TQ:(ch + 1) * TQ, :] = res.results[c]["outT"].T
    if _trace:
        kernel.last_results = res
    return out
